# revision 1
# baseline (speedup 1.0000x reference)
"""Trainium2 Bass kernel for nn_CharEmbedding (ragged_sequence).

Computation (see reference):
    rep = concat([emb[first], emb[mid].sum(1), emb[last]], -1)   # [U, 3H]
    out = rep @ head_w + head_b                                  # [U, O]
    tok = out[inv_i].reshape(B, L, O); pad time by (1,1)         # [B, L+2, O]

Strategy: fuse everything at token granularity, data-parallel over the
B*L = 32768 output tokens (4096 per core = exactly 2 sequences).  Host
precomputes per-token vocab indices (first/mid/last gathered through
inv_i) as int16 in the SWDGE dma_gather wrapped layout.  On each core:

  - emb table (bf16, padded to 4096 rows) resident in SBUF, swizzled for
    SBUF-source transposed dma_gather: partition = id % 128, rank = id // 128.
  - per 512-token tile: TWO dma_gathers of 7*512 = 3584 rows each
    (first+mids0-5 | mids6-11+last), transposed, producing [128, 2, 3584]
    bf16 feature-major (matmul-ready lhsT).  Splitting the gather lets
    Q7 descriptor-gen overlap SDMA drain (measured ~2x faster than one
    7168-row gather; multi-queue spreading raced on HW, so queue 0 only).
  - 12 mid embeddings summed pairwise (tree) on DVE in bf16 (2x mode).
  - PE: out[tok, :] = bias (K=1 matmul of ones x bias) + sum over 6
    K-chunks of embT.T @ W_chunk, accumulated in PSUM (fp32).
  - ACT evacuates PSUM -> SBUF fp32, HWDGE DMA stores to DRAM.

Output rows land contiguously; host assembles the [16, 2050, 768] padded
result (pad rows are zeros and never touch the device).
"""

import numpy as np
import ml_dtypes

import concourse.bacc as bacc
import concourse.mybir as mybir
import concourse.tile as tile
from concourse.bass_utils import run_bass_kernel_spmd

BF16 = ml_dtypes.bfloat16

# Problem constants (hardcoded per contract).
VOCAB = 4000
VOCAB_PAD = 4096
U = 30000
M = 12
H = 256
O = 768
B = 16
L = 2048
N_CORES = 8
T_CORE = (B * L) // N_CORES      # 4096 tokens per core
TILE_T = 512                     # tokens per pipeline tile
ROWS_PER_TOK = 2 + M             # 14 gathered rows per token
KCH = (3 * H) // 128             # 6 K-chunks of the 768-dim contraction
NQ = 1                           # SWDGE queues used for gathers

_NC_CACHE = {}


def build_nc(n_tiles=T_CORE // TILE_T, table_in_sbuf=True, nq=NQ, reps=1,
             gbufs=2, mbufs=2, obufs=4, pbufs=4):
    """Build (and compile) the per-core Bass module.

    Tokens handled = n_tiles * TILE_T.  All cores run the same program.
    reps > 1 wraps the pipeline in a For_i hardware loop (timing only).
    """
    t_core = n_tiles * TILE_T
    rows_tile = ROWS_PER_TOK * TILE_T          # 7168
    rows_half = rows_tile // 2                 # 3584 (= 7 streams of 512)
    idx_cols = rows_tile // 16                 # 448 idx columns per tile
    half_cols = idx_cols // 2

    nc = bacc.Bacc("TRN2", target_bir_lowering=False, debug=False,
                   num_swdge_queues=nq)

    if table_in_sbuf:
        tbl_d = nc.dram_tensor("tbl", [128, (VOCAB_PAD // 128) * H],
                               mybir.dt.bfloat16, kind="ExternalInput")
    else:
        tbl_d = nc.dram_tensor("tbl", [VOCAB_PAD, H], mybir.dt.bfloat16,
                               kind="ExternalInput")
    wts_d = nc.dram_tensor("wts", [128, KCH * O], mybir.dt.bfloat16,
                           kind="ExternalInput")
    bias_d = nc.dram_tensor("bias", [1, O], mybir.dt.bfloat16,
                            kind="ExternalInput")
    idx_d = nc.dram_tensor("idx", [128, n_tiles * idx_cols], mybir.dt.int16,
                           kind="ExternalInput")
    out_d = nc.dram_tensor("out", [t_core, O], mybir.dt.float32,
                           kind="ExternalOutput")

    with tile.TileContext(nc) as tc:
        with (
            tc.tile_pool(name="const", bufs=1) as cpool,
            tc.tile_pool(name="gath", bufs=gbufs) as gpool,
            tc.tile_pool(name="mids", bufs=mbufs) as mpool,
            tc.tile_pool(name="outs", bufs=obufs) as opool,
            tc.tile_pool(name="psum", bufs=pbufs, space="PSUM") as ppool,
        ):
            # ---- resident constants ----
            if table_in_sbuf:
                tbl = cpool.tile([128, (VOCAB_PAD // 128) * H], mybir.dt.bfloat16)
                nc.sync.dma_start(out=tbl[:], in_=tbl_d[:])
            wts = cpool.tile([128, KCH, O], mybir.dt.bfloat16)
            nc.sync.dma_start(out=wts[:], in_=wts_d[:].rearrange(
                "p (c o) -> p c o", c=KCH))
            bias_t = cpool.tile([1, O], mybir.dt.bfloat16)
            nc.sync.dma_start(out=bias_t[:], in_=bias_d[:])
            idx_t = cpool.tile([128, n_tiles * idx_cols], mybir.dt.int16)
            nc.sync.dma_start(out=idx_t[:], in_=idx_d[:])
            ones_t = cpool.tile([1, 128], mybir.dt.bfloat16)
            nc.vector.memset(ones_t[:], 1.0)

            import contextlib
            rep_ctx = tc.For_i(0, reps, 1) if reps > 1 else contextlib.nullcontext()
            with rep_ctx:
             for t in range(n_tiles):
                # ---- gather: 2 x 3584 rows, transposed (feature-major) ----
                # stream order per tile: first, mid0..mid11, last (512 each)
                # g0 = first + mid0..5, g1 = mid6..11 + last
                gs = []
                for h in range(2):
                    gh = gpool.tile([128, 2, rows_half], mybir.dt.bfloat16,
                                    name=f"g{h}", tag=f"g{h}")
                    col0 = t * idx_cols + h * half_cols
                    q = (2 * t + h) % nq
                    if table_in_sbuf:
                        nc.gpsimd.dma_gather(
                            gh[:], tbl[:],
                            idx_t[:, col0:col0 + half_cols],
                            rows_half, rows_half, H,
                            transpose=True,
                            sbuf_tokens_per_rank=128,
                            sbuf_free_dim_per_rank=2 * H,
                            single_packet=False, queue_num=q)
                    else:
                        nc.gpsimd.dma_gather(
                            gh[:], tbl_d[:],
                            idx_t[:, col0:col0 + half_cols],
                            rows_half, rows_half, H,
                            transpose=True,
                            single_packet=False, queue_num=q)
                    gs.append(gh)

                first = gs[0][:, :, 0:TILE_T]
                last = gs[1][:, :, 6 * TILE_T:7 * TILE_T]

                def mid(j):
                    gh = gs[j // 6]
                    off = (1 + j) * TILE_T if j < 6 else (j - 6) * TILE_T
                    return gh[:, :, off:off + TILE_T]

                # ---- mid-sum: pairwise tree on DVE (bf16, 2x mode) ----
                msA = mpool.tile([128, 2, 6 * TILE_T], mybir.dt.bfloat16)
                for k in range(6):
                    nc.vector.tensor_add(
                        msA[:, :, k * TILE_T:(k + 1) * TILE_T],
                        mid(2 * k), mid(2 * k + 1))
                msB = mpool.tile([128, 2, 3 * TILE_T], mybir.dt.bfloat16)
                for k in range(3):
                    nc.vector.tensor_add(
                        msB[:, :, k * TILE_T:(k + 1) * TILE_T],
                        msA[:, :, 2 * k * TILE_T:(2 * k + 1) * TILE_T],
                        msA[:, :, (2 * k + 1) * TILE_T:(2 * k + 2) * TILE_T])
                msum = mpool.tile([128, 2, TILE_T], mybir.dt.bfloat16)
                nc.vector.tensor_add(
                    msum[:], msB[:, :, 0:TILE_T], msB[:, :, TILE_T:2 * TILE_T])
                nc.vector.tensor_add(
                    msum[:], msum[:], msB[:, :, 2 * TILE_T:3 * TILE_T])

                groups = (first, msum[:], last)

                # ---- matmuls + evacuate + store, per 128-token subtile ----
                for m in range(TILE_T // 128):
                    tok = slice(m * 128, (m + 1) * 128)
                    ps_a = ppool.tile([128, 512], mybir.dt.float32)
                    ps_b = ppool.tile([128, 256], mybir.dt.float32)
                    for ps, osl in ((ps_a, slice(0, 512)), (ps_b, slice(512, O))):
                        nc.tensor.matmul(ps[:], ones_t[:], bias_t[:, osl],
                                         start=True, stop=False)
                        for c in range(KCH):
                            gsrc = groups[c // 2]
                            nc.tensor.matmul(
                                ps[:], gsrc[:, c % 2, tok], wts[:, c, osl],
                                start=False, stop=(c == KCH - 1))
                    o_sb = opool.tile([128, O], mybir.dt.float32)
                    nc.scalar.copy(o_sb[:, 0:512], ps_a[:])
                    nc.scalar.copy(o_sb[:, 512:O], ps_b[:])
                    row = t * TILE_T + m * 128
                    nc.sync.dma_start(out=out_d[row:row + 128, :], in_=o_sb[:])

    nc.compile()
    return nc


def _get_nc(n_tiles=T_CORE // TILE_T, table_in_sbuf=True):
    key = (n_tiles, table_in_sbuf)
    if key not in _NC_CACHE:
        _NC_CACHE[key] = build_nc(*key)
    return _NC_CACHE[key]


def _wrap_idx(stream):
    """Pack an index stream into the SWDGE gather layout: idx i lives at
    [i % 16, i // 16], replicated across the 8 groups of 16 partitions."""
    n = stream.shape[0]
    arr = stream.reshape(n // 16, 16).T.astype(np.int16)   # [16, n//16]
    return np.tile(arr, (8, 1))                            # [128, n//16]


def prep_inputs(emb_table, head_w, head_b, first, mid, last, inv_i,
                n_tiles=T_CORE // TILE_T, table_in_sbuf=True):
    """Host-side shard + layout prep.  Returns in_maps for 8 cores."""
    emb = np.asarray(emb_table, dtype=np.float32).copy()
    emb[0] = 0.0  # padding_idx (reference masks id 0; row 0 is zero anyway)
    tbl16 = np.zeros((VOCAB_PAD, H), dtype=BF16)
    tbl16[:VOCAB] = emb.astype(BF16)
    if table_in_sbuf:
        # partition = id % 128, rank (free-dim block) = id // 128
        tbl_in = np.ascontiguousarray(
            tbl16.reshape(VOCAB_PAD // 128, 128, H).transpose(1, 0, 2)
        ).reshape(128, (VOCAB_PAD // 128) * H)
    else:
        tbl_in = tbl16

    Wb = np.asarray(head_w, dtype=np.float32).astype(BF16)      # [768, 768]
    wts_in = np.ascontiguousarray(
        Wb.reshape(KCH, 128, O).transpose(1, 0, 2)).reshape(128, KCH * O)
    bias_in = np.asarray(head_b, dtype=np.float32).astype(BF16).reshape(1, O)

    inv_i = np.asarray(inv_i)
    fi = np.asarray(first)[inv_i].astype(np.int16)   # [B*L]
    mi = np.asarray(mid)[inv_i].astype(np.int16)     # [B*L, 12]
    la = np.asarray(last)[inv_i].astype(np.int16)    # [B*L]

    in_maps = []
    for c in range(N_CORES):
        base = c * T_CORE
        cols = []
        for t in range(n_tiles):
            s = slice(base + t * TILE_T, base + (t + 1) * TILE_T)
            stream = np.concatenate(
                [fi[s]] + [mi[s, j] for j in range(M)] + [la[s]])
            cols.append(_wrap_idx(stream))
        idx_in = np.concatenate(cols, axis=1)
        in_maps.append({
            "tbl": tbl_in, "wts": wts_in, "bias": bias_in, "idx": idx_in,
        })
    return in_maps


def kernel(emb_table, head_w, head_b, first, mid, last, inv_i,
           batch, seq_len, _nc=None, _return_raw=False):
    batch = int(batch)
    seq_len = int(seq_len)
    assert batch == B and seq_len == L, (batch, seq_len)
    nc = _nc if _nc is not None else _get_nc()
    in_maps = prep_inputs(emb_table, head_w, head_b, first, mid, last, inv_i)
    res = run_bass_kernel_spmd(nc, in_maps, core_ids=list(range(N_CORES)))
    per_core = [r["out"] for r in res.results]         # each [4096, 768] f32
    if _return_raw:
        return per_core
    full = np.zeros((B, L + 2, O), dtype=np.float32)
    seq_per_core = T_CORE // L                         # 2 sequences per core
    for c in range(N_CORES):
        full[c * seq_per_core:(c + 1) * seq_per_core, 1:L + 1, :] = (
            per_core[c].reshape(seq_per_core, L, O))
    return full

